# revision 16
# baseline (speedup 1.0000x reference)
"""Multi-head attention (B=2, S=2048, D=1024, H=16) on 8 Trainium2 cores.

Sharding: tensor-parallel over heads — 2 heads per core. Each core:
  - reads full q, k, v [B*S, D] fp32
  - gets pre-transposed weight shards: wqT/wkT/wvT [D, 128] (its 2 heads'
    columns of W^T), biases [128,1], dwT [128, D] (its 128 rows of dense W^T)
  - computes qhT/khT/vhT [128, S] per batch via PE-transpose of inputs +
    float32r matmuls
  - scores are computed transposed, sT[j, i] = kh[j].qh[i], so softmax is a
    free-dim reduction: exp on ScalarE (scale=1/8 fused), the row-of-ones
    appended to vh makes the ctx matmul also produce the softmax sums
  - attn written as attnT[b, h_local, j, i] in fp16 (host transposes + casts)
  - dense partial (its 128 features x full D) written fp16; host sums partials

Host: assemble attn (transpose + fp32 cast), sum op partials + dense bias.
"""

import sys

for _p in ("/opt/trn_rl_repo",):
    if _p not in sys.path:
        sys.path.insert(0, _p)

import numpy as np

import concourse.bass as bass
import concourse.tile as tile
from concourse import mybir
from concourse.masks import make_identity

F32 = mybir.dt.float32
F32R = mybir.dt.float32r
F16 = mybir.dt.float16

# Problem shape (hardcoded per contest rules)
B = 2
S = 2048
D_MODEL = 1024
NUM_HEAD = 16
DEPTH = 64
N_CORES = 8
HEADS_PER_CORE = NUM_HEAD // N_CORES  # 2
FEAT = HEADS_PER_CORE * DEPTH  # 128
SCALE = 1.0 / np.sqrt(DEPTH)  # folded into exp's scale operand


def _patch_tile_drain():
    """This walrus build accepts only one sem-wait per CTRL instruction; the
    Tile kernel-tail drain carries one wait per outstanding producer.  Spread
    them across single-wait NOPs on SP before a wait-free drain."""
    from concourse.vector_clock import ScopedClock

    def _dab(self, tick_clock, wait_clock):
        nc = self.nc
        probe = nc.sync.nop(nofuse=True, hint="tail_wait_probe")
        wait_clock.add_sem_waits(
            probe.ins, ScopedClock({None: tick_clock.global_clock})
        )
        waits = list(probe.ins.sync_info.on_wait or []) if probe.ins.sync_info else []
        if len(waits) > 1:
            probe.ins.sync_info.on_wait = [waits[0]]
            for w in waits[1:]:
                n2 = nc.sync.nop(nofuse=True, hint="tail_wait_extra")
                n2.ins.sync_info = mybir.SyncInfo(on_wait=[w], on_update=[])
        nc.sync.drain()
        nc.all_engine_barrier()
        popped = nc._tile_sem_poison_stack.pop()
        assert popped is self._sem_poison
        nc.clear_and_free_semaphores(list(self.sems.allocated().values()))
        nc.all_engine_barrier()

    tile.TileContext._drain_and_barrier = _dab


_patch_tile_drain()


def _split_multi_waits(nc):
    """This walrus build accepts only one sem-wait per instruction.  Hoist
    extra waits onto same-engine NOPs inserted immediately before the
    carrying instruction (engine streams are in-order, so semantics are
    preserved)."""
    n_added = 0
    for f in nc.m.functions:
        for blk in f.blocks:
            new = []
            for ins in blk.instructions:
                si = ins.sync_info
                if si and si.on_wait and len(si.on_wait) > 1:
                    waits = list(si.on_wait)
                    for i, w in enumerate(waits[:-1]):
                        nop = mybir.InstNoOp(name=f"{ins.name}-xw{i}", ins=[], outs=[])
                        nop.engine = ins.engine
                        nop.sync_info = mybir.SyncInfo(on_wait=[w], on_update=[])
                        new.append(nop)
                        n_added += 1
                    si.on_wait = [waits[-1]]
                new.append(ins)
            blk.instructions = new
    return n_added


def build_nc(s=S, b=B, d_model=D_MODEL, ic=512, split_waits=True):
    """Build the per-core Bass program.  Parameterized so a scaled-down
    variant can run under CoreSim."""
    n_ic = s // ic  # i-chunks per batch
    n_jt = s // 128  # key tiles per batch
    n_sc = s // 1024 if s >= 1024 else 1  # s-chunks for transpose+proj staging
    sc_w = min(s, 1024)  # s-chunk width
    n_st = sc_w // 128  # 128-row tiles per s-chunk
    n_kt = d_model // 128  # contraction tiles for projections
    n_oc = d_model // 512  # dense output chunks
    exp_t = mybir.ActivationFunctionType.Exp
    id_t = mybir.ActivationFunctionType.Identity

    nc = bass.Bass(target_bir_lowering=False)
    qx = nc.dram_tensor("qx", [b * s, d_model], F32R, kind="ExternalInput")
    kx = nc.dram_tensor("kx", [b * s, d_model], F32R, kind="ExternalInput")
    vx = nc.dram_tensor("vx", [b * s, d_model], F32R, kind="ExternalInput")
    wqT = nc.dram_tensor("wqT", [d_model, FEAT], F32R, kind="ExternalInput")
    wkT = nc.dram_tensor("wkT", [d_model, FEAT], F32R, kind="ExternalInput")
    wvT = nc.dram_tensor("wvT", [d_model, FEAT], F32R, kind="ExternalInput")
    bq = nc.dram_tensor("bq", [FEAT, 1], F32, kind="ExternalInput")
    bk = nc.dram_tensor("bk", [FEAT, 1], F32, kind="ExternalInput")
    bv = nc.dram_tensor("bv", [FEAT, 1], F32, kind="ExternalInput")
    dwT = nc.dram_tensor("dwT", [FEAT, d_model], F32R, kind="ExternalInput")
    attnT_d = nc.dram_tensor(
        "attnT", [b, HEADS_PER_CORE, s, s], F16, kind="ExternalOutput"
    )
    op_d = nc.dram_tensor("op", [b * s, d_model], F16, kind="ExternalOutput")

    from contextlib import ExitStack

    with tile.TileContext(nc) as tc, ExitStack() as ctx:
        singles = ctx.enter_context(tc.tile_pool(name="singles", bufs=1))
        sb = ctx.enter_context(tc.tile_pool(name="sb", bufs=2))
        ps = ctx.enter_context(tc.tile_pool(name="ps", bufs=2, space="PSUM"))

        # --- static tiles ---
        ident_f32 = singles.tile([128, 128], F32)
        make_identity(nc, ident_f32)
        ident_r = singles.tile([128, 128], F32R, name="ident_r")
        nc.vector.tensor_copy(ident_r, ident_f32)

        w_sbs = {}
        b_sbs = {}
        for name, wdram, bdram in (
            ("q", wqT, bq),
            ("k", wkT, bk),
            ("v", wvT, bv),
        ):
            w_sb = singles.tile([128, n_kt, FEAT], F32R, name=f"w{name}_sb")
            nc.sync.dma_start(
                out=w_sb, in_=wdram.rearrange("(kt p) m -> p kt m", p=128)
            )
            b_sb = singles.tile([FEAT, 1], F32, name=f"b{name}_sb")
            nc.sync.dma_start(out=b_sb, in_=bdram[:, :])
            w_sbs[name] = w_sb
            b_sbs[name] = b_sb
        dwT_sb = singles.tile([FEAT, d_model], F32R)
        nc.sync.dma_start(out=dwT_sb, in_=dwT[:, :])
        ones_f32 = singles.tile([1, 128], F32)
        nc.vector.memset(ones_f32, 1.0)
        ones_col = singles.tile([1, 128], F32R, name="ones_col")
        nc.vector.tensor_copy(ones_col, ones_f32)

        # persistent per-batch projections (tags reused across b)
        qhT = singles.tile([FEAT, s], F32R, name="qhT")
        khT = singles.tile([FEAT, s], F32R, name="khT")
        vhT = singles.tile([FEAT, s], F32R, name="vhT")
        vhextA = singles.tile([128, n_jt, DEPTH + 1], F16, name="vhextA")
        vhextB = singles.tile([128, n_jt, DEPTH + 1], F16, name="vhextB")

        for bi in range(b):
            # ================= phase A: transpose inputs + project =========
            for tname, xdram, yT in (("k", kx, khT), ("q", qx, qhT), ("v", vx, vhT)):
                w_sb = w_sbs[tname]
                b_sb = b_sbs[tname]
                for sc in range(n_sc):
                    base_row = bi * s + sc * sc_w
                    xT = sb.tile([128, n_kt, sc_w], F32R, tag="xT", bufs=1)
                    for st in range(n_st):
                        xn = sb.tile([128, d_model], F32R, tag="xn", bufs=3)
                        nc.sync.dma_start(
                            out=xn,
                            in_=xdram[base_row + st * 128 : base_row + (st + 1) * 128, :],
                        )
                        for dg in range(n_kt // 4):
                            trp = ps.tile([128, 4, 128], F32R, tag="ps0", bufs=2)
                            for q4 in range(4):
                                dt = dg * 4 + q4
                                nc.tensor.transpose(
                                    trp[:, q4, :],
                                    xn[:, dt * 128 : (dt + 1) * 128],
                                    ident_r,
                                )
                            dst = xT[
                                :, dg * 4 : (dg + 1) * 4, st * 128 : (st + 1) * 128
                            ]
                            if (st + dg) % 2 == 0:
                                nc.vector.tensor_copy(dst, trp)
                            else:
                                nc.scalar.copy(dst, trp)
                    # project this s-chunk: yT[:, s-chunk] = W^T.T @ xT
                    pw = min(512, sc_w)
                    for nch in range(sc_w // pw):
                        ptag = "ps1" if nch % 2 == 0 else "ps2"
                        pr = ps.tile([FEAT, pw], F32, tag=ptag, bufs=2)
                        for kt in range(n_kt):
                            nc.tensor.matmul(
                                pr,
                                w_sb[:, kt, :],
                                xT[:, kt, nch * pw : (nch + 1) * pw],
                                start=(kt == 0),
                                stop=(kt == n_kt - 1),
                            )
                        nc.scalar.activation(
                            yT[:, sc * sc_w + nch * pw : sc * sc_w + (nch + 1) * pw],
                            pr,
                            id_t,
                            bias=b_sb[:, :],
                        )

            # ================= phase A2: vh natural layout + ones column ===
            nc.vector.memset(vhextA[:, :, DEPTH : DEPTH + 1], 1.0)
            nc.vector.memset(vhextB[:, :, DEPTH : DEPTH + 1], 1.0)
            for jt in range(n_jt):
                trp = ps.tile([128, 128], F32R, tag="ps0", bufs=2)
                nc.tensor.transpose(
                    trp,
                    vhT[:, jt * 128 : (jt + 1) * 128],
                    ident_r,
                )
                nc.vector.tensor_copy(vhextA[:, jt, 0:DEPTH], trp[:, 0:DEPTH])
                nc.vector.tensor_copy(vhextB[:, jt, 0:DEPTH], trp[:, DEPTH:FEAT])

            # ================= phase B: attention + dense ==================
            for ici in range(n_ic):
                isl = slice(ici * ic, (ici + 1) * ic)
                ctxA = ps.tile([DEPTH + 1, ic], F32, tag="ctxA", bufs=1)
                ctxB = ps.tile([DEPTH + 1, ic], F32, tag="ctxB", bufs=1)
                pA = sb.tile([128, n_jt, ic], F16, tag="pA", bufs=2)
                pB = sb.tile([128, n_jt, ic], F16, tag="pB", bufs=2)
                # software-pipelined: scores/exp at jt, ctx matmuls at jt-1
                for jt in range(n_jt + 1):
                    if jt < n_jt:
                        jsl = slice(jt * 128, (jt + 1) * 128)
                        sA = ps.tile([128, ic], F32, tag="ps1", bufs=2)
                        sB = ps.tile([128, ic], F32, tag="ps2", bufs=2)
                        nc.tensor.matmul(
                            sA,
                            khT[0:DEPTH, jsl],
                            qhT[0:DEPTH, isl],
                            start=True,
                            stop=True,
                            tile_position=(0, 0),
                        )
                        nc.tensor.matmul(
                            sB,
                            khT[DEPTH:FEAT, jsl],
                            qhT[DEPTH:FEAT, isl],
                            start=True,
                            stop=True,
                            tile_position=(64, 0),
                        )
                        nc.scalar.activation(pA[:, jt, :], sA, exp_t, scale=SCALE)
                        nc.scalar.activation(pB[:, jt, :], sB, exp_t, scale=SCALE)
                    if jt > 0:
                        pj = jt - 1
                        nc.tensor.matmul(
                            ctxA,
                            vhextA[:, pj, :],
                            pA[:, pj, :],
                            start=(pj == 0),
                            stop=(pj == n_jt - 1),
                        )
                        nc.tensor.matmul(
                            ctxB,
                            vhextB[:, pj, :],
                            pB[:, pj, :],
                            start=(pj == 0),
                            stop=(pj == n_jt - 1),
                        )
                # softmax sums -> reciprocals, broadcast across partitions via
                # a K=1 outer-product matmul (ones_col^T @ rinv)
                rinvA32 = sb.tile([1, ic], F32R, tag="riA32", bufs=2)
                rinvB32 = sb.tile([1, ic], F32R, tag="riB32", bufs=2)
                with nc.allow_low_precision(reason="f32r is 4-byte"):
                    nc.vector.reciprocal(rinvA32, ctxA[DEPTH : DEPTH + 1, :])
                    nc.vector.reciprocal(rinvB32, ctxB[DEPTH : DEPTH + 1, :])
                psbcA = ps.tile([128, ic], F32, tag="ps1", bufs=2)
                psbcB = ps.tile([128, ic], F32, tag="ps2", bufs=2)
                nc.tensor.matmul(
                    psbcA,
                    ones_col,
                    rinvA32,
                    start=True,
                    stop=True,
                )
                nc.tensor.matmul(
                    psbcB,
                    ones_col,
                    rinvB32,
                    start=True,
                    stop=True,
                )
                rbcA = sb.tile([128, ic], F16, tag="rbcA", bufs=2)
                rbcB = sb.tile([128, ic], F16, tag="rbcB", bufs=2)
                nc.vector.tensor_copy(rbcA, psbcA)
                nc.vector.tensor_copy(rbcB, psbcB)
                # normalize p in place, then stream out as attnT
                for jt in range(n_jt):
                    nc.vector.tensor_mul(pA[:, jt, :], pA[:, jt, :], rbcA)
                    nc.vector.tensor_mul(pB[:, jt, :], pB[:, jt, :], rbcB)
                outA = attnT_d[bi, 0].rearrange("(jt p) i -> p jt i", p=128)
                outB = attnT_d[bi, 1].rearrange("(jt p) i -> p jt i", p=128)
                nc.sync.dma_start(out=outA[:, :, isl], in_=pA)
                nc.sync.dma_start(out=outB[:, :, isl], in_=pB)
                # ctx eviction + normalization
                ctxsb = sb.tile([FEAT, ic], F32R, tag="ctxsb", bufs=2)
                nc.vector.tensor_copy(ctxsb[0:DEPTH, :], ctxA[0:DEPTH, :])
                nc.vector.tensor_copy(ctxsb[DEPTH:FEAT, :], ctxB[0:DEPTH, :])
                nc.vector.tensor_mul(
                    ctxsb[0:DEPTH, :], ctxsb[0:DEPTH, :], psbcA[0:DEPTH, :]
                )
                nc.vector.tensor_mul(
                    ctxsb[DEPTH:FEAT, :], ctxsb[DEPTH:FEAT, :], psbcB[0:DEPTH, :]
                )
                # dense: op_part[i, :] = ctx^T @ dwT
                for isub in range(ic // 128):
                    opsb = sb.tile([128, d_model], F16, tag="opsb", bufs=2)
                    for oc in range(n_oc):
                        dps = ps.tile([128, 512], F32, tag="ps0", bufs=2)
                        nc.tensor.matmul(
                            dps,
                            ctxsb[:, isub * 128 : (isub + 1) * 128],
                            dwT_sb[:, oc * 512 : (oc + 1) * 512],
                            start=True,
                            stop=True,
                        )
                        nc.vector.tensor_copy(opsb[:, oc * 512 : (oc + 1) * 512], dps)
                    r0 = bi * s + ici * ic + isub * 128
                    nc.sync.dma_start(out=op_d[r0 : r0 + 128, :], in_=opsb)

    if split_waits:
        _split_multi_waits(nc)
    return nc


def shard_inputs(q, k, v, wq_w, wq_b, wk_w, wk_b, wv_w, wv_b, dense_w, dense_b):
    """Host-side prep: full activations to every core, weight shards per core."""
    b, s, d = q.shape
    qx = np.ascontiguousarray(q.reshape(b * s, d), dtype=np.float32)
    kx = np.ascontiguousarray(k.reshape(b * s, d), dtype=np.float32)
    vx = np.ascontiguousarray(v.reshape(b * s, d), dtype=np.float32)
    in_maps = []
    for c in range(N_CORES):
        rows = slice(c * FEAT, (c + 1) * FEAT)
        in_maps.append(
            {
                "qx": qx,
                "kx": kx,
                "vx": vx,
                "wqT": np.ascontiguousarray(wq_w[rows, :].T, dtype=np.float32),
                "wkT": np.ascontiguousarray(wk_w[rows, :].T, dtype=np.float32),
                "wvT": np.ascontiguousarray(wv_w[rows, :].T, dtype=np.float32),
                "bq": np.ascontiguousarray(
                    wq_b[rows].reshape(FEAT, 1), dtype=np.float32
                ),
                "bk": np.ascontiguousarray(
                    wk_b[rows].reshape(FEAT, 1), dtype=np.float32
                ),
                "bv": np.ascontiguousarray(
                    wv_b[rows].reshape(FEAT, 1), dtype=np.float32
                ),
                "dwT": np.ascontiguousarray(dense_w[:, rows].T, dtype=np.float32),
            }
        )
    return in_maps


def assemble_outputs(results, dense_b, b=B, s=S, d=D_MODEL):
    op = np.zeros((b * s, d), dtype=np.float32)
    for r in results:
        op += r["op"].astype(np.float32)
    op += np.asarray(dense_b, dtype=np.float32)[None, :]
    op = op.reshape(b, s, d)
    attn = np.empty((b, NUM_HEAD, s, s), dtype=np.float32)
    for c, r in enumerate(results):
        at = r["attnT"]  # [b, 2, s_j, s_i] fp16
        for bi in range(b):
            for l in range(HEADS_PER_CORE):
                attn[bi, HEADS_PER_CORE * c + l] = at[bi, l].T.astype(np.float32)
    return op, attn


_nc_cache = {}


def _get_nc():
    if "nc" not in _nc_cache:
        _nc_cache["nc"] = build_nc()
    return _nc_cache["nc"]


def kernel(q, k, v, wq_w, wq_b, wk_w, wk_b, wv_w, wv_b, dense_w, dense_b):
    from concourse.bass_utils import run_bass_kernel_spmd

    in_maps = shard_inputs(
        q, k, v, wq_w, wq_b, wk_w, wk_b, wv_w, wv_b, dense_w, dense_b
    )
    res = run_bass_kernel_spmd(_get_nc(), in_maps, core_ids=list(range(N_CORES)))
    return assemble_outputs(res.results, dense_b)
